# revision 20
# baseline (speedup 1.0000x reference)
"""KAN layer (B-spline + SiLU) Trainium2 kernel, v3.

Math: y[b,k] = scale * sum_i( silu(x[b,i])*W[i,k]
                              + sum_j basis_j(x[b,i]) * C[i,k,j] )

u = (x - g0)/h in [0, 11); basis_j(x) = B3(u - j), j = 0..7 (cubic
cardinal B-splines). Two representations are mixed, both bounded (so
the float32r matmul's reduced mantissa does not amplify errors):

  * j < NBUMP: exact bumps via the symmetric-min form. With
    v = relu(min(u-j, 4+j-u)) (in [0,2]):  6*B3 = v^3 - 4*relu(v-1)^3.
    Two custom DVE passes (BUMP_V then BUMP_B; out = v^3/4 - relu(v-1)^3
    = 1.5*B3).
  * j >= NBUMP: Gaussian-CDF approximation of the cumulative spline.
    With phi_s = cumulative B3 smoothstep (0->1 over [s, s+3]),
    B3(u-j) = phi_j - phi_{j+1} and phi_s(u) ~ 0.5*(1+erf(A*(u-s-1.5))),
    A = 1.3817 (sup err 5.2e-3). One ACT-engine Erf pass per phi;
    the 0.5 offsets fold into the weights and a per-k output bias.
    y_spline = sum_s V_s * phi_s with V_s = C_s - C_{s-1} (telescoped),
    so at u >= 11 (host-masked to +16, all phi = 1) the sum is exactly 0.

silu = x * sigmoid(x): Sigmoid on ACT (same act table set as Erf ->
single ACT_TABLE_LOAD), multiply on DVE.

Engines: ACT 8 passes (sigmoid + 7 erf), DVE 5 passes (2x BUMP_V,
2x BUMP_B, silu multiply) + output copy; PE: 10 f32r chunks x 2 PSUM
halves issued in feature-readiness order. fp16 in/out DMA.

Sharding: data-parallel over batch, 1024 rows per core on 8 cores.
"""

import math
import os
import sys

import numpy as np

if "/opt/trn_rl_repo" not in sys.path:
    sys.path.insert(0, "/opt/trn_rl_repo")

import concourse.bass as bass
import concourse.mybir as mybir
from concourse import bacc
from concourse.tile import TileContext

B_TOTAL = 8192
IN_DIM = 128
OUT_DIM = 128
N_CORES = 8
B_CORE = B_TOTAL // N_CORES  # 1024

NB = 8            # num basis functions
NBUMP = 3         # exact DVE bumps j = 0..NBUMP-1
N_ERF = 9 - NBUMP  # phi shifts s = NBUMP..8
NCHUNK = 1 + NBUMP + N_ERF  # silu + bumps + erfs = 10
ERF_A = 1.3817    # erf scale: phi_s(u) ~ 0.5*(1+erf(A*(u-s-1.5)))
U_MASK = 16.0     # u >= 11 masked to this (all phi -> 1, bumps -> 0)

F32 = mybir.dt.float32
F32R = mybir.dt.float32r
F16 = mybir.dt.float16
BF16 = mybir.dt.bfloat16
AF = mybir.ActivationFunctionType
ALU = mybir.AluOpType

# ---------------------------------------------------------------- custom DVE ops


def _register_ops():
    from concourse.dve_ops import (
        _CUSTOM_DVE_ROW_BASE,
        _SUB_OPCODE_FOR_NAME,
        CUSTOM_DVE_SPECS,
        OPS,
        DveOp,
    )
    from concourse.dve_spec import C0, C1, C2, One, Spec, Src0, lower, minn, relu, sq
    from concourse.dve_uop import DveOpSpec

    def reg(name, spec):
        for op in OPS:
            if op.name == name:
                return op
        row = _CUSTOM_DVE_ROW_BASE + len(OPS)
        assert row < 0x20
        _SUB_OPCODE_FOR_NAME[name] = row
        shas = {}
        for ver in ("v3", "v4"):
            s = DveOpSpec(name=name, opcode=row, uops=lower(spec, ver=ver),
                          rd1_en=False)
            shas[ver] = s.sha(ver)
        op = DveOp(name, spec, subdim=False, uops_sha=shas)
        OPS.append(op)
        CUSTOM_DVE_SPECS[name] = spec
        return op

    # v' = relu(min(u + C0, C1 - u))   (C0 = -j, C1 = 4+j)
    bump_v = Spec(
        body=relu(minn(Src0 + C0, C1 - Src0)),
        reference=lambda in0, in1, s0, s1, imm2: np.maximum(
            np.minimum(in0.astype(np.float32) + s0,
                       s1 - in0.astype(np.float32)), 0.0
        ).astype(np.float32),
    )
    # out = v^3 * C2 - relu((v-1)^3)   (C2 = imm2 = 0.25; v >= 0)
    _v = Src0
    _w = _v - One
    bump_b = Spec(
        body=(sq(_v) * _v) * C2 - relu(sq(_w) * _w),
        reference=lambda in0, in1, s0, s1, imm2: (
            in0.astype(np.float32) ** 3 * imm2
            - np.maximum(in0.astype(np.float32) - 1.0, 0.0) ** 3
        ).astype(np.float32),
    )
    return reg("ANT_KAN_BUMP_V", bump_v), reg("ANT_KAN_BUMP_B", bump_b)


OP_BUMP_V, OP_BUMP_B = _register_ops()

# ---------------------------------------------------------------- device kernel

_NC_CACHE = {}

# feat chunk layout: 0 = silu, 1..NBUMP = bumps j=0..NBUMP-1,
# 1+NBUMP+e = erf shift s = NBUMP+e (e = 0..N_ERF-1)


def _build_nc():
    if "nc" in _NC_CACHE:
        return _NC_CACHE["nc"]
    nc = bacc.Bacc("TRN2", target_bir_lowering=False)

    # Register the erf scale/bias scalars as SBUF const APs, memset on the
    # gpsimd queue right after the framework's own const memsets (~6us).
    # First consumer (ACT erf) runs >3us later, after the uT DMA lands.
    def _register_const(value):
        key = (F32, float(value))
        if key not in nc.const_aps.aps:
            t = nc.alloc_sbuf_tensor(f"const-f32-{value}", [128, 1], F32)
            nc.gpsimd.memset(t.ap(), float(value))
            nc.const_aps.aps[key] = t.ap()

    _register_const(ERF_A)
    for e in range(N_ERF):
        _register_const(-ERF_A * (NBUMP + e + 1.5))

    xT = nc.dram_tensor("xT", [IN_DIM, B_CORE], F16, kind="ExternalInput")
    uT = nc.dram_tensor("uT", [IN_DIM, B_CORE], F16, kind="ExternalInput")
    cb = nc.dram_tensor("cb", [IN_DIM, 12], F32, kind="ExternalInput")
    wf = nc.dram_tensor("wf", [IN_DIM, NCHUNK, OUT_DIM], BF16, kind="ExternalInput")
    yT = nc.dram_tensor("yT", [OUT_DIM, B_CORE], F16, kind="ExternalOutput")

    with TileContext(nc) as tc:
        with (
            tc.tile_pool(name="dpool", bufs=1) as dpool,
            tc.tile_pool(name="ppool", bufs=2, space="PSUM") as ppool,
        ):
            # x and u alone on the sync HWDGE ring (fastest arrival); consts
            # and weights via the gpsimd SWDGE queue (slower, loose deadlines).
            ut = dpool.tile([IN_DIM, B_CORE], F16, tag="ut")
            nc.sync.dma_start(out=ut[:], in_=uT[:])
            xt = dpool.tile([IN_DIM, B_CORE], F16, tag="xt")
            nc.sync.dma_start(out=xt[:], in_=xT[:])
            wt = dpool.tile([IN_DIM, NCHUNK, OUT_DIM], BF16, tag="wt")
            nc.gpsimd.dma_start(out=wt[:], in_=wf[:])
            cbt = dpool.tile([IN_DIM, 12], F32, tag="cbt")
            nc.gpsimd.dma_start(out=cbt[:], in_=cb[:])

            # PE p-state warmup: a stream of small matmuls on the weight
            # tile, bridging from the weight DMA completion (~11.3us) into
            # the first feature-gated matmul so the PE clock ramps early.
            pwarm = ppool.tile([IN_DIM, 128], F32, tag="pwarm")
            for _ in range(10):
                nc.tensor.matmul(pwarm[:], lhsT=wt[:, 0, :],
                                 rhs=wt[:, 0, 0:128],
                                 start=True, stop=True, skip_group_check=True)

            feat = dpool.tile([IN_DIM, NCHUNK, B_CORE], BF16, tag="feat")
            sg = dpool.tile([IN_DIM, B_CORE], BF16, tag="sg")
            vt = dpool.tile([IN_DIM, NBUMP, B_CORE], F32, tag="vt")

            # ---- ACT: u arrives first -> two erfs, sigmoid at x-arrival, rest
            def _erf(e):
                nc.scalar.activation(
                    feat[:, 1 + NBUMP + e, :], ut[:], AF.Erf,
                    bias=-ERF_A * (NBUMP + e + 1.5), scale=ERF_A,
                )
            _erf(0)
            _erf(1)
            nc.scalar.activation(sg[:], xt[:], AF.Sigmoid)
            for e in range(2, N_ERF):
                _erf(e)

            # ---- DVE: bump v/b passes interleaved with silu multiply
            nc.vector._custom_dve(OP_BUMP_V, out=vt[:, 0, :], in0=ut[:],
                                  s0=0.0, s1=4.0)
            nc.vector._custom_dve(OP_BUMP_B, out=feat[:, 1, :], in0=vt[:, 0, :],
                                  imm2=0.25)
            nc.vector.tensor_tensor(out=feat[:, 0, :], in0=xt[:], in1=sg[:],
                                    op=ALU.mult)
            for j in range(1, NBUMP):
                nc.vector._custom_dve(OP_BUMP_V, out=vt[:, j, :], in0=ut[:],
                                      s0=-float(j), s1=float(4 + j))
                nc.vector._custom_dve(OP_BUMP_B, out=feat[:, 1 + j, :],
                                      in0=vt[:, j, :], imm2=0.25)

            # ---- PE: warmup matmuls (p-state ramp) then chunk accumulation
            # readiness order: DVE bump0 ~2.4, erf3 ~2.5, silu ~3.1, erf4 ~3.7,
            # erf5 ~4.9, bump1 ~5.5, erf6 ~6.2, erf7 ~7.4, bump2 ~7.9, erf8 ~8.6
            order = [4, 5, 1, 0, 6, 2, 7, 8, 3, 9]
            assert sorted(order) == list(range(NCHUNK))
            ps0 = ppool.tile([OUT_DIM, 512], F32, tag="ps0")
            ps1 = ppool.tile([OUT_DIM, 512], F32, tag="ps1")
            ps = [ps0, ps1]

            for ci, c in enumerate(order):
                for h2 in range(2):
                    nc.tensor.matmul(
                        ps[h2][:],
                        lhsT=wt[:, c, :],
                        rhs=feat[:, c, h2 * 512:(h2 + 1) * 512],
                        start=(ci == 0),
                        stop=(ci == NCHUNK - 1),
                    )

            # ---- copies + ybias (ACT half 0 || DVE half 1), -> fp16, DMA out
            yt = dpool.tile([OUT_DIM, B_CORE], F16, tag="yt")
            nc.scalar.activation(yt[:, 0:512], ps0[:], AF.Identity,
                                 bias=cbt[:, 9:10])
            nc.vector.tensor_scalar(out=yt[:, 512:1024], in0=ps1[:],
                                    scalar1=cbt[:, 9:10], scalar2=None,
                                    op0=ALU.add)
            nc.scalar.dma_start(out=yT[:, 0:512], in_=yt[:, 0:512])
            nc.sync.dma_start(out=yT[:, 512:1024], in_=yt[:, 512:1024])

    nc.finalize()
    _NC_CACHE["nc"] = nc
    return nc


# ---------------------------------------------------------------- host wrapper


def _build_weights(grid, spline_coeff, base_weight, scale):
    g0 = float(grid[0, 0])
    h = float(grid[0, 1] - grid[0, 0])
    sc = float(np.asarray(scale).reshape(-1)[0])
    C = np.asarray(spline_coeff, dtype=np.float64)      # [i, k, j]
    W = np.asarray(base_weight, dtype=np.float64)       # [i, k]

    import ml_dtypes
    wfull = np.empty((IN_DIM, NCHUNK, OUT_DIM), dtype=ml_dtypes.bfloat16)
    wfull[:, 0, :] = (W * sc).astype(np.float32)
    for j in range(NBUMP):
        # bump op emits 1.5*B3 -> weight C_j * (2/3)
        wfull[:, 1 + j, :] = (C[:, :, j] * (2.0 / 3.0) * sc).astype(np.float32)
    Vsum = np.zeros((IN_DIM, OUT_DIM), dtype=np.float64)
    for e in range(N_ERF):
        s = NBUMP + e
        Vs = np.zeros((IN_DIM, OUT_DIM), dtype=np.float64)
        if s <= NB - 1:
            Vs += C[:, :, s]
        if s - 1 >= NBUMP:
            Vs -= C[:, :, s - 1]
        Vsum += Vs
        # erf feature is raw erf in [-1,1]; phi = 0.5 + 0.5*erf
        wfull[:, 1 + NBUMP + e, :] = (0.5 * Vs * sc).astype(np.float32)
    ybias = (0.5 * Vsum.sum(axis=0) * sc).astype(np.float32)    # [k]

    cb = np.zeros((IN_DIM, 12), dtype=np.float32)
    cb[:, 0] = ERF_A
    for e in range(N_ERF):
        s = NBUMP + e
        cb[:, 1 + e] = -ERF_A * (s + 1.5)
    cb[:, 9] = ybias
    return wfull, cb, g0, h


def _prepare_host_inputs(x, grid, spline_coeff, base_weight, scale):
    wfull, cb, g0, h = _build_weights(grid, spline_coeff, base_weight, scale)
    x = np.asarray(x, dtype=np.float32)
    u = x / h - g0 / h
    u = np.where(u < float(NB + 3), u, U_MASK).astype(np.float32)
    xT16 = np.ascontiguousarray(x.T.astype(np.float16))        # [128, 8192]
    uT16 = np.ascontiguousarray(u.T.astype(np.float16))
    in_maps = []
    for c in range(N_CORES):
        sl = slice(c * B_CORE, (c + 1) * B_CORE)
        in_maps.append({
            "xT": np.ascontiguousarray(xT16[:, sl]),
            "uT": np.ascontiguousarray(uT16[:, sl]),
            "cb": cb,
            "wf": wfull,
        })
    return in_maps


def kernel(x, grid, spline_coeff, base_weight, scale):
    from concourse.bass_utils import run_bass_kernel_spmd

    in_maps = _prepare_host_inputs(x, grid, spline_coeff, base_weight, scale)
    nc = _build_nc()
    res = run_bass_kernel_spmd(nc, in_maps, core_ids=list(range(N_CORES)))
    outs = res.results
    yT = np.concatenate([outs[c]["yT"] for c in range(N_CORES)], axis=1)
    return np.ascontiguousarray(yT.T.astype(np.float32))


if __name__ == "__main__":
    rng = np.random.default_rng(0)
    x = rng.standard_normal((B_TOTAL, IN_DIM)).astype(np.float32)
    g = np.linspace(-1, 1, 6)
    hh = 0.4
    for _ in range(3):
        g = np.concatenate([[g[0] - hh], g, [g[-1] + hh]])
    grid = np.broadcast_to(g.astype(np.float32), (IN_DIM, 12)).copy()
    C = rng.standard_normal((IN_DIM, OUT_DIM, NB)).astype(np.float32)
    W = rng.standard_normal((IN_DIM, OUT_DIM)).astype(np.float32)
    s = np.ones((1,), np.float32)
    y = kernel(x, grid, C, W, s)
    print(y.shape, y.dtype, np.abs(y).max())


# revision 21
# speedup vs baseline: 1.1046x; 1.1046x over previous
"""KAN layer (B-spline + SiLU) Trainium2 kernel, v3.

Math: y[b,k] = scale * sum_i( silu(x[b,i])*W[i,k]
                              + sum_j basis_j(x[b,i]) * C[i,k,j] )

u = (x - g0)/h in [0, 11); basis_j(x) = B3(u - j), j = 0..7 (cubic
cardinal B-splines). Two representations are mixed, both bounded (so
the float32r matmul's reduced mantissa does not amplify errors):

  * j < NBUMP: exact bumps via the symmetric-min form. With
    v = relu(min(u-j, 4+j-u)) (in [0,2]):  6*B3 = v^3 - 4*relu(v-1)^3.
    Two custom DVE passes (BUMP_V then BUMP_B; out = v^3/4 - relu(v-1)^3
    = 1.5*B3).
  * j >= NBUMP: Gaussian-CDF approximation of the cumulative spline.
    With phi_s = cumulative B3 smoothstep (0->1 over [s, s+3]),
    B3(u-j) = phi_j - phi_{j+1} and phi_s(u) ~ 0.5*(1+erf(A*(u-s-1.5))),
    A = 1.3817 (sup err 5.2e-3). One ACT-engine Erf pass per phi;
    the 0.5 offsets fold into the weights and a per-k output bias.
    y_spline = sum_s V_s * phi_s with V_s = C_s - C_{s-1} (telescoped),
    so at u >= 11 (host-masked to +16, all phi = 1) the sum is exactly 0.

silu = x * sigmoid(x): Sigmoid on ACT (same act table set as Erf ->
single ACT_TABLE_LOAD), multiply on DVE.

Engines: ACT 8 passes (sigmoid + 7 erf), DVE 5 passes (2x BUMP_V,
2x BUMP_B, silu multiply) + output copy; PE: 10 f32r chunks x 2 PSUM
halves issued in feature-readiness order. fp16 in/out DMA.

Sharding: data-parallel over batch, 1024 rows per core on 8 cores.
"""

import math
import os
import sys

import numpy as np

if "/opt/trn_rl_repo" not in sys.path:
    sys.path.insert(0, "/opt/trn_rl_repo")

import concourse.bass as bass
import concourse.mybir as mybir
from concourse import bacc
from concourse.tile import TileContext

B_TOTAL = 8192
IN_DIM = 128
OUT_DIM = 128
N_CORES = 8
B_CORE = B_TOTAL // N_CORES  # 1024

NB = 8            # num basis functions
NBUMP = 3         # exact DVE bumps j = 0..NBUMP-1
N_ERF = 9 - NBUMP  # phi shifts s = NBUMP..8
NCHUNK = 1 + NBUMP + N_ERF  # silu + bumps + erfs = 10
ERF_A = 1.3817    # erf scale: phi_s(u) ~ 0.5*(1+erf(A*(u-s-1.5)))
U_MASK = 16.0     # u >= 11 masked to this (all phi -> 1, bumps -> 0)

F32 = mybir.dt.float32
F32R = mybir.dt.float32r
F16 = mybir.dt.float16
BF16 = mybir.dt.bfloat16
AF = mybir.ActivationFunctionType
ALU = mybir.AluOpType

# ---------------------------------------------------------------- custom DVE ops


def _register_ops():
    from concourse.dve_ops import (
        _CUSTOM_DVE_ROW_BASE,
        _SUB_OPCODE_FOR_NAME,
        CUSTOM_DVE_SPECS,
        OPS,
        DveOp,
    )
    from concourse.dve_spec import C0, C1, C2, One, Spec, Src0, lower, minn, relu, sq
    from concourse.dve_uop import DveOpSpec

    def reg(name, spec):
        for op in OPS:
            if op.name == name:
                return op
        row = _CUSTOM_DVE_ROW_BASE + len(OPS)
        assert row < 0x20
        _SUB_OPCODE_FOR_NAME[name] = row
        shas = {}
        for ver in ("v3", "v4"):
            s = DveOpSpec(name=name, opcode=row, uops=lower(spec, ver=ver),
                          rd1_en=False)
            shas[ver] = s.sha(ver)
        op = DveOp(name, spec, subdim=False, uops_sha=shas)
        OPS.append(op)
        CUSTOM_DVE_SPECS[name] = spec
        return op

    # v' = relu(min(u + C0, C1 - u))   (C0 = -j, C1 = 4+j)
    bump_v = Spec(
        body=relu(minn(Src0 + C0, C1 - Src0)),
        reference=lambda in0, in1, s0, s1, imm2: np.maximum(
            np.minimum(in0.astype(np.float32) + s0,
                       s1 - in0.astype(np.float32)), 0.0
        ).astype(np.float32),
    )
    # out = v^3 * C2 - relu((v-1)^3)   (C2 = imm2 = 0.25; v >= 0)
    _v = Src0
    _w = _v - One
    bump_b = Spec(
        body=(sq(_v) * _v) * C2 - relu(sq(_w) * _w),
        reference=lambda in0, in1, s0, s1, imm2: (
            in0.astype(np.float32) ** 3 * imm2
            - np.maximum(in0.astype(np.float32) - 1.0, 0.0) ** 3
        ).astype(np.float32),
    )
    return reg("ANT_KAN_BUMP_V", bump_v), reg("ANT_KAN_BUMP_B", bump_b)


OP_BUMP_V, OP_BUMP_B = _register_ops()

# ---------------------------------------------------------------- device kernel

_NC_CACHE = {}

# feat chunk layout: 0 = silu, 1..NBUMP = bumps j=0..NBUMP-1,
# 1+NBUMP+e = erf shift s = NBUMP+e (e = 0..N_ERF-1)


def _build_nc():
    if "nc" in _NC_CACHE:
        return _NC_CACHE["nc"]
    nc = bacc.Bacc("TRN2", target_bir_lowering=False)

    # Register the erf scale/bias scalars as SBUF const APs, memset on the
    # gpsimd queue right after the framework's own const memsets (~6us).
    # First consumer (ACT erf) runs >3us later, after the uT DMA lands.
    def _register_const(value):
        key = (F32, float(value))
        if key not in nc.const_aps.aps:
            t = nc.alloc_sbuf_tensor(f"const-f32-{value}", [128, 1], F32)
            nc.gpsimd.memset(t.ap(), float(value))
            nc.const_aps.aps[key] = t.ap()

    _register_const(ERF_A)
    for e in range(N_ERF):
        _register_const(-ERF_A * (NBUMP + e + 1.5))

    xT = nc.dram_tensor("xT", [IN_DIM, B_CORE], F16, kind="ExternalInput")
    uT = nc.dram_tensor("uT", [IN_DIM, B_CORE], F16, kind="ExternalInput")
    cb = nc.dram_tensor("cb", [IN_DIM, 12], F32, kind="ExternalInput")
    wf = nc.dram_tensor("wf", [IN_DIM, NCHUNK, OUT_DIM], BF16, kind="ExternalInput")
    yT = nc.dram_tensor("yT", [OUT_DIM, B_CORE], F16, kind="ExternalOutput")

    with TileContext(nc) as tc:
        with (
            tc.tile_pool(name="dpool", bufs=1) as dpool,
            tc.tile_pool(name="ppool", bufs=2, space="PSUM") as ppool,
        ):
            # x and u alone on the sync HWDGE ring (fastest arrival); consts
            # and weights via the gpsimd SWDGE queue (slower, loose deadlines).
            ut = dpool.tile([IN_DIM, B_CORE], F16, tag="ut")
            nc.sync.dma_start(out=ut[:], in_=uT[:])
            wt = dpool.tile([IN_DIM, NCHUNK, OUT_DIM], BF16, tag="wt")
            nc.sync.dma_start(out=wt[:], in_=wf[:])
            xt = dpool.tile([IN_DIM, B_CORE], F16, tag="xt")
            nc.sync.dma_start(out=xt[:], in_=xT[:])
            cbt = dpool.tile([IN_DIM, 12], F32, tag="cbt")
            nc.gpsimd.dma_start(out=cbt[:], in_=cb[:])

            # PE p-state warmup: a stream of small matmuls on the weight
            # tile, bridging from the weight DMA completion (~11.3us) into
            # the first feature-gated matmul so the PE clock ramps early.
            pwarm = ppool.tile([IN_DIM, 128], F32, tag="pwarm")
            for _ in range(10):
                nc.tensor.matmul(pwarm[:], lhsT=wt[:, 0, :],
                                 rhs=wt[:, 0, 0:128],
                                 start=True, stop=True, skip_group_check=True)

            feat = dpool.tile([IN_DIM, NCHUNK, B_CORE], BF16, tag="feat")
            sg = dpool.tile([IN_DIM, B_CORE], BF16, tag="sg")
            vt = dpool.tile([IN_DIM, NBUMP, B_CORE], F32, tag="vt")

            # ---- ACT: u arrives first -> two erfs, sigmoid at x-arrival, rest
            def _erf(e):
                nc.scalar.activation(
                    feat[:, 1 + NBUMP + e, :], ut[:], AF.Erf,
                    bias=-ERF_A * (NBUMP + e + 1.5), scale=ERF_A,
                )
            _erf(0)
            _erf(1)
            nc.scalar.activation(sg[:], xt[:], AF.Sigmoid)
            for e in range(2, N_ERF):
                _erf(e)

            # ---- DVE: bump v/b passes interleaved with silu multiply
            for j in range(NBUMP):
                nc.vector._custom_dve(OP_BUMP_V, out=vt[:, j, :], in0=ut[:],
                                      s0=-float(j), s1=float(4 + j))
                nc.vector._custom_dve(OP_BUMP_B, out=feat[:, 1 + j, :],
                                      in0=vt[:, j, :], imm2=0.25)
                if j == 1:
                    nc.vector.tensor_tensor(out=feat[:, 0, :], in0=xt[:],
                                            in1=sg[:], op=ALU.mult)

            # ---- PE: warmup matmuls (p-state ramp) then chunk accumulation
            # readiness order: DVE bump0 ~2.4, erf3 ~2.5, silu ~3.1, erf4 ~3.7,
            # erf5 ~4.9, bump1 ~5.5, erf6 ~6.2, erf7 ~7.4, bump2 ~7.9, erf8 ~8.6
            order = [4, 5, 1, 6, 2, 0, 7, 8, 9, 3]
            assert sorted(order) == list(range(NCHUNK))
            ps0 = ppool.tile([OUT_DIM, 512], F32, tag="ps0")
            ps1 = ppool.tile([OUT_DIM, 512], F32, tag="ps1")
            ps = [ps0, ps1]

            for ci, c in enumerate(order):
                for h2 in range(2):
                    nc.tensor.matmul(
                        ps[h2][:],
                        lhsT=wt[:, c, :],
                        rhs=feat[:, c, h2 * 512:(h2 + 1) * 512],
                        start=(ci == 0),
                        stop=(ci == NCHUNK - 1),
                    )

            # ---- copies + ybias (ACT half 0 || DVE half 1), -> fp16, DMA out
            yt = dpool.tile([OUT_DIM, B_CORE], F16, tag="yt")
            nc.scalar.activation(yt[:, 0:512], ps0[:], AF.Identity,
                                 bias=cbt[:, 9:10])
            nc.vector.tensor_scalar(out=yt[:, 512:1024], in0=ps1[:],
                                    scalar1=cbt[:, 9:10], scalar2=None,
                                    op0=ALU.add)
            nc.scalar.dma_start(out=yT[:, 0:512], in_=yt[:, 0:512])
            nc.sync.dma_start(out=yT[:, 512:1024], in_=yt[:, 512:1024])

    nc.finalize()
    _NC_CACHE["nc"] = nc
    return nc


# ---------------------------------------------------------------- host wrapper


def _build_weights(grid, spline_coeff, base_weight, scale):
    g0 = float(grid[0, 0])
    h = float(grid[0, 1] - grid[0, 0])
    sc = float(np.asarray(scale).reshape(-1)[0])
    C = np.asarray(spline_coeff, dtype=np.float64)      # [i, k, j]
    W = np.asarray(base_weight, dtype=np.float64)       # [i, k]

    import ml_dtypes
    wfull = np.empty((IN_DIM, NCHUNK, OUT_DIM), dtype=ml_dtypes.bfloat16)
    wfull[:, 0, :] = (W * sc).astype(np.float32)
    for j in range(NBUMP):
        # bump op emits 1.5*B3 -> weight C_j * (2/3)
        wfull[:, 1 + j, :] = (C[:, :, j] * (2.0 / 3.0) * sc).astype(np.float32)
    Vsum = np.zeros((IN_DIM, OUT_DIM), dtype=np.float64)
    for e in range(N_ERF):
        s = NBUMP + e
        Vs = np.zeros((IN_DIM, OUT_DIM), dtype=np.float64)
        if s <= NB - 1:
            Vs += C[:, :, s]
        if s - 1 >= NBUMP:
            Vs -= C[:, :, s - 1]
        Vsum += Vs
        # erf feature is raw erf in [-1,1]; phi = 0.5 + 0.5*erf
        wfull[:, 1 + NBUMP + e, :] = (0.5 * Vs * sc).astype(np.float32)
    ybias = (0.5 * Vsum.sum(axis=0) * sc).astype(np.float32)    # [k]

    cb = np.zeros((IN_DIM, 12), dtype=np.float32)
    cb[:, 0] = ERF_A
    for e in range(N_ERF):
        s = NBUMP + e
        cb[:, 1 + e] = -ERF_A * (s + 1.5)
    cb[:, 9] = ybias
    return wfull, cb, g0, h


def _prepare_host_inputs(x, grid, spline_coeff, base_weight, scale):
    wfull, cb, g0, h = _build_weights(grid, spline_coeff, base_weight, scale)
    x = np.asarray(x, dtype=np.float32)
    u = x / h - g0 / h
    u = np.where(u < float(NB + 3), u, U_MASK).astype(np.float32)
    xT16 = np.ascontiguousarray(x.T.astype(np.float16))        # [128, 8192]
    uT16 = np.ascontiguousarray(u.T.astype(np.float16))
    in_maps = []
    for c in range(N_CORES):
        sl = slice(c * B_CORE, (c + 1) * B_CORE)
        in_maps.append({
            "xT": np.ascontiguousarray(xT16[:, sl]),
            "uT": np.ascontiguousarray(uT16[:, sl]),
            "cb": cb,
            "wf": wfull,
        })
    return in_maps


def kernel(x, grid, spline_coeff, base_weight, scale):
    from concourse.bass_utils import run_bass_kernel_spmd

    in_maps = _prepare_host_inputs(x, grid, spline_coeff, base_weight, scale)
    nc = _build_nc()
    res = run_bass_kernel_spmd(nc, in_maps, core_ids=list(range(N_CORES)))
    outs = res.results
    yT = np.concatenate([outs[c]["yT"] for c in range(N_CORES)], axis=1)
    return np.ascontiguousarray(yT.T.astype(np.float32))


if __name__ == "__main__":
    rng = np.random.default_rng(0)
    x = rng.standard_normal((B_TOTAL, IN_DIM)).astype(np.float32)
    g = np.linspace(-1, 1, 6)
    hh = 0.4
    for _ in range(3):
        g = np.concatenate([[g[0] - hh], g, [g[-1] + hh]])
    grid = np.broadcast_to(g.astype(np.float32), (IN_DIM, 12)).copy()
    C = rng.standard_normal((IN_DIM, OUT_DIM, NB)).astype(np.float32)
    W = rng.standard_normal((IN_DIM, OUT_DIM)).astype(np.float32)
    s = np.ones((1,), np.float32)
    y = kernel(x, grid, C, W, s)
    print(y.shape, y.dtype, np.abs(y).max())


# revision 22
# speedup vs baseline: 1.1517x; 1.0427x over previous
"""KAN layer (B-spline + SiLU) Trainium2 kernel, v3.

Math: y[b,k] = scale * sum_i( silu(x[b,i])*W[i,k]
                              + sum_j basis_j(x[b,i]) * C[i,k,j] )

u = (x - g0)/h in [0, 11); basis_j(x) = B3(u - j), j = 0..7 (cubic
cardinal B-splines). Two representations are mixed, both bounded (so
the float32r matmul's reduced mantissa does not amplify errors):

  * j < NBUMP: exact bumps via the symmetric-min form. With
    v = relu(min(u-j, 4+j-u)) (in [0,2]):  6*B3 = v^3 - 4*relu(v-1)^3.
    Two custom DVE passes (BUMP_V then BUMP_B; out = v^3/4 - relu(v-1)^3
    = 1.5*B3).
  * j >= NBUMP: Gaussian-CDF approximation of the cumulative spline.
    With phi_s = cumulative B3 smoothstep (0->1 over [s, s+3]),
    B3(u-j) = phi_j - phi_{j+1} and phi_s(u) ~ 0.5*(1+erf(A*(u-s-1.5))),
    A = 1.3817 (sup err 5.2e-3). One ACT-engine Erf pass per phi;
    the 0.5 offsets fold into the weights and a per-k output bias.
    y_spline = sum_s V_s * phi_s with V_s = C_s - C_{s-1} (telescoped),
    so at u >= 11 (host-masked to +16, all phi = 1) the sum is exactly 0.

silu = x * sigmoid(x): Sigmoid on ACT (same act table set as Erf ->
single ACT_TABLE_LOAD), multiply on DVE.

Engines: ACT 8 passes (sigmoid + 7 erf), DVE 5 passes (2x BUMP_V,
2x BUMP_B, silu multiply) + output copy; PE: 10 f32r chunks x 2 PSUM
halves issued in feature-readiness order. fp16 in/out DMA.

Sharding: data-parallel over batch, 1024 rows per core on 8 cores.
"""

import math
import os
import sys

import numpy as np

if "/opt/trn_rl_repo" not in sys.path:
    sys.path.insert(0, "/opt/trn_rl_repo")

import concourse.bass as bass
import concourse.mybir as mybir
from concourse import bacc
from concourse.tile import TileContext

B_TOTAL = 8192
IN_DIM = 128
OUT_DIM = 128
N_CORES = 8
B_CORE = B_TOTAL // N_CORES  # 1024

NB = 8            # num basis functions
NBUMP = 3         # exact DVE bumps j = 0..NBUMP-1
N_ERF = 9 - NBUMP  # phi shifts s = NBUMP..8
NCHUNK = 1 + NBUMP + N_ERF  # silu + bumps + erfs = 10
ERF_A = 1.3817    # erf scale: phi_s(u) ~ 0.5*(1+erf(A*(u-s-1.5)))
U_MASK = 16.0     # u >= 11 masked to this (all phi -> 1, bumps -> 0)

F32 = mybir.dt.float32
F32R = mybir.dt.float32r
F16 = mybir.dt.float16
BF16 = mybir.dt.bfloat16
AF = mybir.ActivationFunctionType
ALU = mybir.AluOpType

# ---------------------------------------------------------------- custom DVE ops


def _register_ops():
    from concourse.dve_ops import (
        _CUSTOM_DVE_ROW_BASE,
        _SUB_OPCODE_FOR_NAME,
        CUSTOM_DVE_SPECS,
        OPS,
        DveOp,
    )
    from concourse.dve_spec import C0, C1, C2, One, Spec, Src0, lower, minn, relu, sq
    from concourse.dve_uop import DveOpSpec

    def reg(name, spec):
        for op in OPS:
            if op.name == name:
                return op
        row = _CUSTOM_DVE_ROW_BASE + len(OPS)
        assert row < 0x20
        _SUB_OPCODE_FOR_NAME[name] = row
        shas = {}
        for ver in ("v3", "v4"):
            s = DveOpSpec(name=name, opcode=row, uops=lower(spec, ver=ver),
                          rd1_en=False)
            shas[ver] = s.sha(ver)
        op = DveOp(name, spec, subdim=False, uops_sha=shas)
        OPS.append(op)
        CUSTOM_DVE_SPECS[name] = spec
        return op

    # v' = relu(min(u + C0, C1 - u))   (C0 = -j, C1 = 4+j)
    bump_v = Spec(
        body=relu(minn(Src0 + C0, C1 - Src0)),
        reference=lambda in0, in1, s0, s1, imm2: np.maximum(
            np.minimum(in0.astype(np.float32) + s0,
                       s1 - in0.astype(np.float32)), 0.0
        ).astype(np.float32),
    )
    # out = v^3 * C2 - relu((v-1)^3)   (C2 = imm2 = 0.25; v >= 0)
    _v = Src0
    _w = _v - One
    bump_b = Spec(
        body=(sq(_v) * _v) * C2 - relu(sq(_w) * _w),
        reference=lambda in0, in1, s0, s1, imm2: (
            in0.astype(np.float32) ** 3 * imm2
            - np.maximum(in0.astype(np.float32) - 1.0, 0.0) ** 3
        ).astype(np.float32),
    )
    return reg("ANT_KAN_BUMP_V", bump_v), reg("ANT_KAN_BUMP_B", bump_b)


OP_BUMP_V, OP_BUMP_B = _register_ops()

# ---------------------------------------------------------------- device kernel

_NC_CACHE = {}

# feat chunk layout: 0 = silu, 1..NBUMP = bumps j=0..NBUMP-1,
# 1+NBUMP+e = erf shift s = NBUMP+e (e = 0..N_ERF-1)


def _build_nc():
    if "nc" in _NC_CACHE:
        return _NC_CACHE["nc"]
    nc = bacc.Bacc("TRN2", target_bir_lowering=False)

    # Register the erf scale/bias scalars as SBUF const APs, memset on the
    # gpsimd queue right after the framework's own const memsets (~6us).
    # First consumer (ACT erf) runs >3us later, after the uT DMA lands.
    def _register_const(value):
        key = (F32, float(value))
        if key not in nc.const_aps.aps:
            t = nc.alloc_sbuf_tensor(f"const-f32-{value}", [128, 1], F32)
            nc.gpsimd.memset(t.ap(), float(value))
            nc.const_aps.aps[key] = t.ap()

    _register_const(ERF_A)
    for e in range(N_ERF):
        _register_const(-ERF_A * (NBUMP + e + 1.5))

    xT = nc.dram_tensor("xT", [IN_DIM, B_CORE], F16, kind="ExternalInput")
    uT = nc.dram_tensor("uT", [IN_DIM, B_CORE], F16, kind="ExternalInput")
    cb = nc.dram_tensor("cb", [IN_DIM, 12], F32, kind="ExternalInput")
    wf = nc.dram_tensor("wf", [IN_DIM, NCHUNK, OUT_DIM], BF16, kind="ExternalInput")
    yT = nc.dram_tensor("yT", [OUT_DIM, B_CORE], F16, kind="ExternalOutput")

    with TileContext(nc) as tc:
        with (
            tc.tile_pool(name="dpool", bufs=1) as dpool,
            tc.tile_pool(name="ppool", bufs=2, space="PSUM") as ppool,
        ):
            # x and u alone on the sync HWDGE ring (fastest arrival); consts
            # and weights via the gpsimd SWDGE queue (slower, loose deadlines).
            ut = dpool.tile([IN_DIM, B_CORE], F16, tag="ut")
            nc.sync.dma_start(out=ut[:], in_=uT[:])
            wt = dpool.tile([IN_DIM, NCHUNK, OUT_DIM], BF16, tag="wt")
            nc.sync.dma_start(out=wt[:], in_=wf[:])
            xt = dpool.tile([IN_DIM, B_CORE], F16, tag="xt")
            nc.sync.dma_start(out=xt[:], in_=xT[:])
            cbt = dpool.tile([IN_DIM, 12], F32, tag="cbt")
            nc.gpsimd.dma_start(out=cbt[:], in_=cb[:])

            # PE p-state warmup: a stream of small matmuls on the weight
            # tile, bridging from the weight DMA completion (~11.3us) into
            # the first feature-gated matmul so the PE clock ramps early.
            pwarm = ppool.tile([IN_DIM, 128], F32, tag="pwarm")
            for _ in range(10):
                nc.tensor.matmul(pwarm[:], lhsT=wt[:, 0, :],
                                 rhs=wt[:, 0, 0:128],
                                 start=True, stop=True, skip_group_check=True)

            feat = dpool.tile([IN_DIM, NCHUNK, B_CORE], BF16, tag="feat")
            sg = dpool.tile([IN_DIM, B_CORE], BF16, tag="sg")
            vt = dpool.tile([IN_DIM, NBUMP, B_CORE], F32, tag="vt")

            # ---- ACT: u arrives first -> two erfs, sigmoid at x-arrival, rest
            def _erf(e):
                nc.scalar.activation(
                    feat[:, 1 + NBUMP + e, :], ut[:], AF.Erf,
                    bias=-ERF_A * (NBUMP + e + 1.5), scale=ERF_A,
                )
            _erf(0)
            _erf(1)
            _erf(2)
            nc.scalar.activation(sg[:], xt[:], AF.Sigmoid)
            for e in range(3, N_ERF):
                _erf(e)

            # ---- DVE: bump v/b passes interleaved with silu multiply
            for j in range(NBUMP):
                nc.vector._custom_dve(OP_BUMP_V, out=vt[:, j, :], in0=ut[:],
                                      s0=-float(j), s1=float(4 + j))
                nc.vector._custom_dve(OP_BUMP_B, out=feat[:, 1 + j, :],
                                      in0=vt[:, j, :], imm2=0.25)
                if j == 1:
                    nc.vector.tensor_tensor(out=feat[:, 0, :], in0=xt[:],
                                            in1=sg[:], op=ALU.mult)

            # ---- PE: warmup matmuls (p-state ramp) then chunk accumulation
            # readiness order: DVE bump0 ~2.4, erf3 ~2.5, silu ~3.1, erf4 ~3.7,
            # erf5 ~4.9, bump1 ~5.5, erf6 ~6.2, erf7 ~7.4, bump2 ~7.9, erf8 ~8.6
            order = [4, 5, 1, 6, 2, 0, 7, 8, 9, 3]
            assert sorted(order) == list(range(NCHUNK))
            ps0 = ppool.tile([OUT_DIM, 512], F32, tag="ps0")
            ps1 = ppool.tile([OUT_DIM, 512], F32, tag="ps1")
            ps = [ps0, ps1]

            for ci, c in enumerate(order):
                for h2 in range(2):
                    nc.tensor.matmul(
                        ps[h2][:],
                        lhsT=wt[:, c, :],
                        rhs=feat[:, c, h2 * 512:(h2 + 1) * 512],
                        start=(ci == 0),
                        stop=(ci == NCHUNK - 1),
                    )

            # ---- copies + ybias (ACT half 0 || DVE half 1), -> fp16, DMA out
            yt = dpool.tile([OUT_DIM, B_CORE], F16, tag="yt")
            nc.scalar.activation(yt[:, 0:512], ps0[:], AF.Identity,
                                 bias=cbt[:, 9:10])
            nc.vector.tensor_scalar(out=yt[:, 512:1024], in0=ps1[:],
                                    scalar1=cbt[:, 9:10], scalar2=None,
                                    op0=ALU.add)
            nc.scalar.dma_start(out=yT[:, 0:512], in_=yt[:, 0:512])
            nc.sync.dma_start(out=yT[:, 512:1024], in_=yt[:, 512:1024])

    nc.finalize()
    _NC_CACHE["nc"] = nc
    return nc


# ---------------------------------------------------------------- host wrapper


def _build_weights(grid, spline_coeff, base_weight, scale):
    g0 = float(grid[0, 0])
    h = float(grid[0, 1] - grid[0, 0])
    sc = float(np.asarray(scale).reshape(-1)[0])
    C = np.asarray(spline_coeff, dtype=np.float64)      # [i, k, j]
    W = np.asarray(base_weight, dtype=np.float64)       # [i, k]

    import ml_dtypes
    wfull = np.empty((IN_DIM, NCHUNK, OUT_DIM), dtype=ml_dtypes.bfloat16)
    wfull[:, 0, :] = (W * sc).astype(np.float32)
    for j in range(NBUMP):
        # bump op emits 1.5*B3 -> weight C_j * (2/3)
        wfull[:, 1 + j, :] = (C[:, :, j] * (2.0 / 3.0) * sc).astype(np.float32)
    Vsum = np.zeros((IN_DIM, OUT_DIM), dtype=np.float64)
    for e in range(N_ERF):
        s = NBUMP + e
        Vs = np.zeros((IN_DIM, OUT_DIM), dtype=np.float64)
        if s <= NB - 1:
            Vs += C[:, :, s]
        if s - 1 >= NBUMP:
            Vs -= C[:, :, s - 1]
        Vsum += Vs
        # erf feature is raw erf in [-1,1]; phi = 0.5 + 0.5*erf
        wfull[:, 1 + NBUMP + e, :] = (0.5 * Vs * sc).astype(np.float32)
    ybias = (0.5 * Vsum.sum(axis=0) * sc).astype(np.float32)    # [k]

    cb = np.zeros((IN_DIM, 12), dtype=np.float32)
    cb[:, 0] = ERF_A
    for e in range(N_ERF):
        s = NBUMP + e
        cb[:, 1 + e] = -ERF_A * (s + 1.5)
    cb[:, 9] = ybias
    return wfull, cb, g0, h


def _prepare_host_inputs(x, grid, spline_coeff, base_weight, scale):
    wfull, cb, g0, h = _build_weights(grid, spline_coeff, base_weight, scale)
    x = np.asarray(x, dtype=np.float32)
    u = x / h - g0 / h
    u = np.where(u < float(NB + 3), u, U_MASK).astype(np.float32)
    xT16 = np.ascontiguousarray(x.T.astype(np.float16))        # [128, 8192]
    uT16 = np.ascontiguousarray(u.T.astype(np.float16))
    in_maps = []
    for c in range(N_CORES):
        sl = slice(c * B_CORE, (c + 1) * B_CORE)
        in_maps.append({
            "xT": np.ascontiguousarray(xT16[:, sl]),
            "uT": np.ascontiguousarray(uT16[:, sl]),
            "cb": cb,
            "wf": wfull,
        })
    return in_maps


def kernel(x, grid, spline_coeff, base_weight, scale):
    from concourse.bass_utils import run_bass_kernel_spmd

    in_maps = _prepare_host_inputs(x, grid, spline_coeff, base_weight, scale)
    nc = _build_nc()
    res = run_bass_kernel_spmd(nc, in_maps, core_ids=list(range(N_CORES)))
    outs = res.results
    yT = np.concatenate([outs[c]["yT"] for c in range(N_CORES)], axis=1)
    return np.ascontiguousarray(yT.T.astype(np.float32))


if __name__ == "__main__":
    rng = np.random.default_rng(0)
    x = rng.standard_normal((B_TOTAL, IN_DIM)).astype(np.float32)
    g = np.linspace(-1, 1, 6)
    hh = 0.4
    for _ in range(3):
        g = np.concatenate([[g[0] - hh], g, [g[-1] + hh]])
    grid = np.broadcast_to(g.astype(np.float32), (IN_DIM, 12)).copy()
    C = rng.standard_normal((IN_DIM, OUT_DIM, NB)).astype(np.float32)
    W = rng.standard_normal((IN_DIM, OUT_DIM)).astype(np.float32)
    s = np.ones((1,), np.float32)
    y = kernel(x, grid, C, W, s)
    print(y.shape, y.dtype, np.abs(y).max())


# revision 23
# speedup vs baseline: 1.1723x; 1.0179x over previous
"""KAN layer (B-spline + SiLU) Trainium2 kernel, v3.

Math: y[b,k] = scale * sum_i( silu(x[b,i])*W[i,k]
                              + sum_j basis_j(x[b,i]) * C[i,k,j] )

u = (x - g0)/h in [0, 11); basis_j(x) = B3(u - j), j = 0..7 (cubic
cardinal B-splines). Two representations are mixed, both bounded (so
the float32r matmul's reduced mantissa does not amplify errors):

  * j < NBUMP: exact bumps via the symmetric-min form. With
    v = relu(min(u-j, 4+j-u)) (in [0,2]):  6*B3 = v^3 - 4*relu(v-1)^3.
    Two custom DVE passes (BUMP_V then BUMP_B; out = v^3/4 - relu(v-1)^3
    = 1.5*B3).
  * j >= NBUMP: Gaussian-CDF approximation of the cumulative spline.
    With phi_s = cumulative B3 smoothstep (0->1 over [s, s+3]),
    B3(u-j) = phi_j - phi_{j+1} and phi_s(u) ~ 0.5*(1+erf(A*(u-s-1.5))),
    A = 1.3817 (sup err 5.2e-3). One ACT-engine Erf pass per phi;
    the 0.5 offsets fold into the weights and a per-k output bias.
    y_spline = sum_s V_s * phi_s with V_s = C_s - C_{s-1} (telescoped),
    so at u >= 11 (host-masked to +16, all phi = 1) the sum is exactly 0.

silu = x * sigmoid(x): Sigmoid on ACT (same act table set as Erf ->
single ACT_TABLE_LOAD), multiply on DVE.

Engines: ACT 8 passes (sigmoid + 7 erf), DVE 5 passes (2x BUMP_V,
2x BUMP_B, silu multiply) + output copy; PE: 10 f32r chunks x 2 PSUM
halves issued in feature-readiness order. fp16 in/out DMA.

Sharding: data-parallel over batch, 1024 rows per core on 8 cores.
"""

import math
import os
import sys

import numpy as np

if "/opt/trn_rl_repo" not in sys.path:
    sys.path.insert(0, "/opt/trn_rl_repo")

import concourse.bass as bass
import concourse.mybir as mybir
from concourse import bacc
from concourse.tile import TileContext

B_TOTAL = 8192
IN_DIM = 128
OUT_DIM = 128
N_CORES = 8
B_CORE = B_TOTAL // N_CORES  # 1024

NB = 8            # num basis functions
NBUMP = 3         # exact DVE bumps j = 0..NBUMP-1
N_ERF = 9 - NBUMP  # phi shifts s = NBUMP..8
NCHUNK = 1 + NBUMP + N_ERF  # silu + bumps + erfs = 10
ERF_A = 1.3817    # erf scale: phi_s(u) ~ 0.5*(1+erf(A*(u-s-1.5)))
U_MASK = 16.0     # u >= 11 masked to this (all phi -> 1, bumps -> 0)

F32 = mybir.dt.float32
F32R = mybir.dt.float32r
F16 = mybir.dt.float16
BF16 = mybir.dt.bfloat16
AF = mybir.ActivationFunctionType
ALU = mybir.AluOpType

# ---------------------------------------------------------------- custom DVE ops


def _register_ops():
    from concourse.dve_ops import (
        _CUSTOM_DVE_ROW_BASE,
        _SUB_OPCODE_FOR_NAME,
        CUSTOM_DVE_SPECS,
        OPS,
        DveOp,
    )
    from concourse.dve_spec import C0, C1, C2, One, Spec, Src0, lower, minn, relu, sq
    from concourse.dve_uop import DveOpSpec

    def reg(name, spec):
        for op in OPS:
            if op.name == name:
                return op
        row = _CUSTOM_DVE_ROW_BASE + len(OPS)
        assert row < 0x20
        _SUB_OPCODE_FOR_NAME[name] = row
        shas = {}
        for ver in ("v3", "v4"):
            s = DveOpSpec(name=name, opcode=row, uops=lower(spec, ver=ver),
                          rd1_en=False)
            shas[ver] = s.sha(ver)
        op = DveOp(name, spec, subdim=False, uops_sha=shas)
        OPS.append(op)
        CUSTOM_DVE_SPECS[name] = spec
        return op

    # v' = relu(min(u + C0, C1 - u))   (C0 = -j, C1 = 4+j)
    bump_v = Spec(
        body=relu(minn(Src0 + C0, C1 - Src0)),
        reference=lambda in0, in1, s0, s1, imm2: np.maximum(
            np.minimum(in0.astype(np.float32) + s0,
                       s1 - in0.astype(np.float32)), 0.0
        ).astype(np.float32),
    )
    # out = v^3 * C2 - relu((v-1)^3)   (C2 = imm2 = 0.25; v >= 0)
    _v = Src0
    _w = _v - One
    bump_b = Spec(
        body=(sq(_v) * _v) * C2 - relu(sq(_w) * _w),
        reference=lambda in0, in1, s0, s1, imm2: (
            in0.astype(np.float32) ** 3 * imm2
            - np.maximum(in0.astype(np.float32) - 1.0, 0.0) ** 3
        ).astype(np.float32),
    )
    return reg("ANT_KAN_BUMP_V", bump_v), reg("ANT_KAN_BUMP_B", bump_b)


OP_BUMP_V, OP_BUMP_B = _register_ops()

# ---------------------------------------------------------------- device kernel

_NC_CACHE = {}

# feat chunk layout: 0 = silu, 1..NBUMP = bumps j=0..NBUMP-1,
# 1+NBUMP+e = erf shift s = NBUMP+e (e = 0..N_ERF-1)


def _build_nc():
    if "nc" in _NC_CACHE:
        return _NC_CACHE["nc"]
    nc = bacc.Bacc("TRN2", target_bir_lowering=False)

    # Register the erf scale/bias scalars as SBUF const APs, memset on the
    # gpsimd queue right after the framework's own const memsets (~6us).
    # First consumer (ACT erf) runs >3us later, after the uT DMA lands.
    def _register_const(value):
        key = (F32, float(value))
        if key not in nc.const_aps.aps:
            t = nc.alloc_sbuf_tensor(f"const-f32-{value}", [128, 1], F32)
            nc.gpsimd.memset(t.ap(), float(value))
            nc.const_aps.aps[key] = t.ap()

    _register_const(ERF_A)
    for e in range(N_ERF):
        _register_const(-ERF_A * (NBUMP + e + 1.5))

    xT = nc.dram_tensor("xT", [IN_DIM, B_CORE], F16, kind="ExternalInput")
    uT = nc.dram_tensor("uT", [IN_DIM, B_CORE], F16, kind="ExternalInput")
    cb = nc.dram_tensor("cb", [IN_DIM, 12], F32, kind="ExternalInput")
    wf = nc.dram_tensor("wf", [IN_DIM, NCHUNK, OUT_DIM], BF16, kind="ExternalInput")
    yT = nc.dram_tensor("yT", [OUT_DIM, B_CORE], F16, kind="ExternalOutput")

    with TileContext(nc) as tc:
        with (
            tc.tile_pool(name="dpool", bufs=1) as dpool,
            tc.tile_pool(name="ppool", bufs=2, space="PSUM") as ppool,
        ):
            # x and u alone on the sync HWDGE ring (fastest arrival); consts
            # and weights via the gpsimd SWDGE queue (slower, loose deadlines).
            ut = dpool.tile([IN_DIM, B_CORE], F16, tag="ut")
            nc.sync.dma_start(out=ut[:], in_=uT[:])
            wt = dpool.tile([IN_DIM, NCHUNK, OUT_DIM], BF16, tag="wt")
            nc.sync.dma_start(out=wt[:], in_=wf[:])
            xt = dpool.tile([IN_DIM, B_CORE], F16, tag="xt")
            nc.sync.dma_start(out=xt[:], in_=xT[:])
            cbt = dpool.tile([IN_DIM, 12], F32, tag="cbt")
            nc.gpsimd.dma_start(out=cbt[:], in_=cb[:])

            # PE p-state warmup: a stream of small matmuls on the weight
            # tile, bridging from the weight DMA completion (~11.3us) into
            # the first feature-gated matmul so the PE clock ramps early.
            pwarm = ppool.tile([IN_DIM, 128], F32, tag="pwarm")
            for _ in range(10):
                nc.tensor.matmul(pwarm[:], lhsT=wt[:, 0, :],
                                 rhs=wt[:, 0, 0:128],
                                 start=True, stop=True, skip_group_check=True)

            feat = dpool.tile([IN_DIM, NCHUNK, B_CORE], BF16, tag="feat")
            sg = dpool.tile([IN_DIM, B_CORE], BF16, tag="sg")
            vt = dpool.tile([IN_DIM, NBUMP, B_CORE], F32, tag="vt")

            # ---- ACT: u arrives first -> two erfs, sigmoid at x-arrival, rest
            def _erf(e):
                nc.scalar.activation(
                    feat[:, 1 + NBUMP + e, :], ut[:], AF.Erf,
                    bias=-ERF_A * (NBUMP + e + 1.5), scale=ERF_A,
                )
            _erf(0)
            _erf(1)
            _erf(2)
            nc.scalar.activation(sg[:], xt[:], AF.Sigmoid)
            for e in range(3, N_ERF - 1):
                _erf(e)
            # last erf split into halves so the PE can start on h0 early
            eL = N_ERF - 1
            for h2 in range(2):
                nc.scalar.activation(
                    feat[:, 1 + NBUMP + eL, h2 * 512:(h2 + 1) * 512],
                    ut[:, h2 * 512:(h2 + 1) * 512], AF.Erf,
                    bias=-ERF_A * (NBUMP + eL + 1.5), scale=ERF_A,
                )

            # ---- DVE: bump v/b passes interleaved with silu multiply
            for j in range(NBUMP):
                nc.vector._custom_dve(OP_BUMP_V, out=vt[:, j, :], in0=ut[:],
                                      s0=-float(j), s1=float(4 + j))
                if j < NBUMP - 1:
                    nc.vector._custom_dve(OP_BUMP_B, out=feat[:, 1 + j, :],
                                          in0=vt[:, j, :], imm2=0.25)
                else:
                    for h2 in range(2):
                        nc.vector._custom_dve(
                            OP_BUMP_B,
                            out=feat[:, 1 + j, h2 * 512:(h2 + 1) * 512],
                            in0=vt[:, j, h2 * 512:(h2 + 1) * 512], imm2=0.25)
                if j == 1:
                    nc.vector.tensor_tensor(out=feat[:, 0, :], in0=xt[:],
                                            in1=sg[:], op=ALU.mult)

            # ---- PE: warmup matmuls (p-state ramp) then chunk accumulation
            # readiness order: DVE bump0 ~2.4, erf3 ~2.5, silu ~3.1, erf4 ~3.7,
            # erf5 ~4.9, bump1 ~5.5, erf6 ~6.2, erf7 ~7.4, bump2 ~7.9, erf8 ~8.6
            order = [4, 5, 1, 6, 2, 0, 7, 8, 9, 3]
            assert sorted(order) == list(range(NCHUNK))
            ps0 = ppool.tile([OUT_DIM, 512], F32, tag="ps0")
            ps1 = ppool.tile([OUT_DIM, 512], F32, tag="ps1")
            ps = [ps0, ps1]
            tail2 = order[-2:]

            def _mm(c, h2, start, stop):
                nc.tensor.matmul(
                    ps[h2][:],
                    lhsT=wt[:, c, :],
                    rhs=feat[:, c, h2 * 512:(h2 + 1) * 512],
                    start=start, stop=stop,
                )
            for ci, c in enumerate(order[:-2]):
                for h2 in range(2):
                    _mm(c, h2, ci == 0, False)
            # last two chunks: both h0 matmuls first so ps0 closes early
            _mm(tail2[0], 0, False, False)
            _mm(tail2[1], 0, False, True)
            _mm(tail2[0], 1, False, False)
            _mm(tail2[1], 1, False, True)

            # ---- copies + ybias (ACT half 0 || DVE half 1), -> fp16, DMA out
            yt = dpool.tile([OUT_DIM, B_CORE], F16, tag="yt")
            nc.scalar.activation(yt[:, 0:512], ps0[:], AF.Identity,
                                 bias=cbt[:, 9:10])
            nc.vector.tensor_scalar(out=yt[:, 512:1024], in0=ps1[:],
                                    scalar1=cbt[:, 9:10], scalar2=None,
                                    op0=ALU.add)
            nc.scalar.dma_start(out=yT[:, 0:512], in_=yt[:, 0:512])
            nc.sync.dma_start(out=yT[:, 512:1024], in_=yt[:, 512:1024])

    nc.finalize()
    _NC_CACHE["nc"] = nc
    return nc


# ---------------------------------------------------------------- host wrapper


def _build_weights(grid, spline_coeff, base_weight, scale):
    g0 = float(grid[0, 0])
    h = float(grid[0, 1] - grid[0, 0])
    sc = float(np.asarray(scale).reshape(-1)[0])
    C = np.asarray(spline_coeff, dtype=np.float64)      # [i, k, j]
    W = np.asarray(base_weight, dtype=np.float64)       # [i, k]

    import ml_dtypes
    wfull = np.empty((IN_DIM, NCHUNK, OUT_DIM), dtype=ml_dtypes.bfloat16)
    wfull[:, 0, :] = (W * sc).astype(np.float32)
    for j in range(NBUMP):
        # bump op emits 1.5*B3 -> weight C_j * (2/3)
        wfull[:, 1 + j, :] = (C[:, :, j] * (2.0 / 3.0) * sc).astype(np.float32)
    Vsum = np.zeros((IN_DIM, OUT_DIM), dtype=np.float64)
    for e in range(N_ERF):
        s = NBUMP + e
        Vs = np.zeros((IN_DIM, OUT_DIM), dtype=np.float64)
        if s <= NB - 1:
            Vs += C[:, :, s]
        if s - 1 >= NBUMP:
            Vs -= C[:, :, s - 1]
        Vsum += Vs
        # erf feature is raw erf in [-1,1]; phi = 0.5 + 0.5*erf
        wfull[:, 1 + NBUMP + e, :] = (0.5 * Vs * sc).astype(np.float32)
    ybias = (0.5 * Vsum.sum(axis=0) * sc).astype(np.float32)    # [k]

    cb = np.zeros((IN_DIM, 12), dtype=np.float32)
    cb[:, 0] = ERF_A
    for e in range(N_ERF):
        s = NBUMP + e
        cb[:, 1 + e] = -ERF_A * (s + 1.5)
    cb[:, 9] = ybias
    return wfull, cb, g0, h


def _prepare_host_inputs(x, grid, spline_coeff, base_weight, scale):
    wfull, cb, g0, h = _build_weights(grid, spline_coeff, base_weight, scale)
    x = np.asarray(x, dtype=np.float32)
    u = x / h - g0 / h
    u = np.where(u < float(NB + 3), u, U_MASK).astype(np.float32)
    xT16 = np.ascontiguousarray(x.T.astype(np.float16))        # [128, 8192]
    uT16 = np.ascontiguousarray(u.T.astype(np.float16))
    in_maps = []
    for c in range(N_CORES):
        sl = slice(c * B_CORE, (c + 1) * B_CORE)
        in_maps.append({
            "xT": np.ascontiguousarray(xT16[:, sl]),
            "uT": np.ascontiguousarray(uT16[:, sl]),
            "cb": cb,
            "wf": wfull,
        })
    return in_maps


def kernel(x, grid, spline_coeff, base_weight, scale):
    from concourse.bass_utils import run_bass_kernel_spmd

    in_maps = _prepare_host_inputs(x, grid, spline_coeff, base_weight, scale)
    nc = _build_nc()
    res = run_bass_kernel_spmd(nc, in_maps, core_ids=list(range(N_CORES)))
    outs = res.results
    yT = np.concatenate([outs[c]["yT"] for c in range(N_CORES)], axis=1)
    return np.ascontiguousarray(yT.T.astype(np.float32))


if __name__ == "__main__":
    rng = np.random.default_rng(0)
    x = rng.standard_normal((B_TOTAL, IN_DIM)).astype(np.float32)
    g = np.linspace(-1, 1, 6)
    hh = 0.4
    for _ in range(3):
        g = np.concatenate([[g[0] - hh], g, [g[-1] + hh]])
    grid = np.broadcast_to(g.astype(np.float32), (IN_DIM, 12)).copy()
    C = rng.standard_normal((IN_DIM, OUT_DIM, NB)).astype(np.float32)
    W = rng.standard_normal((IN_DIM, OUT_DIM)).astype(np.float32)
    s = np.ones((1,), np.float32)
    y = kernel(x, grid, C, W, s)
    print(y.shape, y.dtype, np.abs(y).max())
